# revision 56
# baseline (speedup 1.0000x reference)
"""Trainium2 Bass kernel for the BINN convnet problem (fp16, transposed layout).

Computation (per row b of inp, all column indices mod D=128):
    g[b, j]  = c1[j] * a[b, j+1] - c2[j] * a[b, j-2]
    x[b, j]  = g[b, j] * a[b, j-1]
    out      = x + a @ W_lin.T + b_lin
with c1[j] = w[j,0]*w[j,2], c2[j] = w[j,1]*w[j,2], except j==1 where the
outer factor is w[1,0] instead of w[1,2].  g is linear in a: g = a @ G.T for
a constant banded G.

The correctness gate is scale-relative absmax < 2e-2; fp16 end-to-end
measures ~7e-4, so all HBM traffic runs at 2 bytes/elem — half the fp32
baseline (32 MiB/core instead of 64 MiB/core; DMA is the roofline).

Strategy: pure data parallel across 8 NeuronCores (batch split).  The host
pre-transposes each 65536-row shard to A^T [128, 65536] fp16, so on-device:

  1. Loads are plain contiguous DMAs (16 KiB per partition per tile),
     no transposes anywhere on device.
  2. g^T = G @ A^T and mm^T = W_lin @ A^T are matmuls with *constant*
     stationary operands (G^T, W_lin^T) and A^T chunks moving, N=512 per
     PSUM bank, natural transposed output layout.
  3. The stencil roll j-1 is absorbed into the constants: the device
     computes the row-rotated output out_dev[p] = out[(p+1) mod 128] using
     rolled G, W_lin, b_lin, so the DVE multiply x_dev = g_rot * A^T is
     perfectly partition-aligned (PSUM partition offsets are illegal) and
     needs no wrap op.  The host un-rotates for free on assembly.
  4. The W matmul accumulates mm on top of x in PSUM (start=False).
     PSUM "zero pending" bits make a bank's first matmul write after
     start=True overwrite instead of accumulate; since the x banks never
     see start=True, their initial pending state is whatever the previous
     NEFF left, which corrupted each bank's first chunk.  A dummy
     full-region start=True matmul per x bank at init clears the bits.
  5. ScalarE evacuates PSUM -> SBUF fp16 adding b_lin, which in transposed
     layout is a per-partition activation bias.
  6. fp16 stores; the host transposes back and upcasts to fp32.  Stores are
     emitted on the ACT queue two 2-chunk groups late so their semaphore
     wait never stalls ACT's queue ahead of pending evacuations.

PE sees [G,G,W,W] per 2-chunk group - one stationary reload per operand
per group; GpSimd stays idle (its fp16 SBUF add measured only ~58 G
elem/s, which made it the bottleneck when it held the final add).
"""

import os
import sys

import numpy as np

if os.path.isdir("/opt/trn_rl_repo") and "/opt/trn_rl_repo" not in sys.path:
    sys.path.insert(0, "/opt/trn_rl_repo")

import concourse.mybir as mybir
import concourse.tile as tile
from concourse import bacc
from concourse.bass_utils import run_bass_kernel_spmd

D = 128          # feature dim
N_CORES = 8
CHUNK = 512      # columns (= batch rows) per PSUM bank / matmul
TCOLS = 8192     # columns per DMA tile (2 MiB fp16)
F16 = mybir.dt.float16
F32 = mybir.dt.float32


def build_program(ncols: int):
    """Build the single-core Bass program (SPMD across cores).

    ncols = rows of the original problem handled by this core; the device
    works on A^T [128, ncols] fp16.
    """
    assert ncols % TCOLS == 0
    ntiles = ncols // TCOLS
    cpt = TCOLS // CHUNK          # chunks per tile (16)
    nchunks = ntiles * cpt

    nc = bacc.Bacc("TRN2", debug=False, target_bir_lowering=False)

    at_d = nc.declare_dram_parameter("at", [D, ncols], F16, isOutput=False)
    gt_d = nc.declare_dram_parameter("gt", [D, D], F16, isOutput=False)
    wt_d = nc.declare_dram_parameter("wt", [D, D], F16, isOutput=False)
    b_d = nc.declare_dram_parameter("b", [D, 1], F32, isOutput=False)
    out_d = nc.declare_dram_parameter("out", [D, ncols], F16, isOutput=True)

    with tile.TileContext(nc) as tc:
        HT = TCOLS // 2  # loads/stores split in 1 MiB halves (ramp/tail)
        with (
            tc.tile_pool(name="const", bufs=1) as const_pool,
            tc.tile_pool(name="a_sb", bufs=4) as a_pool,
            tc.tile_pool(name="o_sb", bufs=4) as o_pool,
            tc.tile_pool(name="g_ps", bufs=4, space="PSUM") as g_pool,
            tc.tile_pool(name="x_ps", bufs=4, space="PSUM") as x_pool,
        ):
            gt_sb = const_pool.tile([D, D], F16)
            wt_sb = const_pool.tile([D, D], F16)
            b_sb = const_pool.tile([D, 1], F32)
            dum_sb = const_pool.tile([1, D + CHUNK], F16)
            # memset on the otherwise-idle GpSimd queue: it is ready ~1 us
            # before DVE's first post-preamble slot, and the warmup matmuls
            # below gate G(0) on the PE queue
            nc.gpsimd.memset(dum_sb[:], 0.0)
            # The first input piece is the head-of-pipeline gate (transfer +
            # ~2us HBM receipt): issue a small first piece FIRST on the SP
            # queue, ahead of the tiny const DMAs, then ramp piece sizes.
            # SP issue order tuned so each piece's landing (issue + transfer
            # + ~2us HBM receipt) leads the 622ns/chunk demand schedule:
            # gt before G(0)~10.5, wt before W(0)~13.9, b before evac(0)~14.3,
            # doubling piece sizes just ahead of their consuming chunks.
            at0_sb = a_pool.tile([D, TCOLS], F16, tag="at")

            def load0(lo, hi):
                nc.sync.dma_start(out=at0_sb[:, lo:hi], in_=at_d[:, lo:hi])

            load0(0, 512)
            nc.sync.dma_start(out=gt_sb[:], in_=gt_d[:, :])
            load0(512, 1024)
            load0(1024, 2048)
            load0(2048, 3072)
            nc.sync.dma_start(out=wt_sb[:], in_=wt_d[:, :])
            nc.sync.dma_start(out=b_sb[:], in_=b_d[:, :])
            load0(3072, 4096)
            load0(4096, 6144)
            load0(6144, 8192)
            # hoist ScalarE's lazy activation-table load out of the pipeline
            warm_sb = const_pool.tile([1, 1], F32)
            nc.scalar.add(out=warm_sb[:], in_=b_sb[0:1, 0:1], add=b_sb[0:1, 0:1])

            # Clear the x banks' PSUM zero-pending bits: one full-region
            # start=True matmul per bank (values are overwritten later).
            # Both operands come from the memset tile so the warmups never
            # wait on the constant DMAs.
            for _ in range(4):
                x_ps = x_pool.tile([D, CHUNK], F32, tag="x")
                nc.tensor.matmul(
                    out=x_ps[:],
                    lhsT=dum_sb[0:1, 0:D],
                    rhs=dum_sb[0:1, D : D + CHUNK],
                    start=True,
                    stop=True,
                )

            tiles = {}  # tile t -> (at_sb, o_sb)
            st = {}     # chunk k -> (at_sb, o_sb, col, x_ps)

            o0_sb = o_pool.tile([D, TCOLS], F16, tag="o")
            tiles[0] = (at0_sb, o0_sb)

            def tile_of(k):
                t, c = divmod(k, cpt)
                if c == 0 and t not in tiles:
                    at_sb = a_pool.tile([D, TCOLS], F16, tag="at")
                    for h in range(2):
                        nc.sync.dma_start(
                            out=at_sb[:, h * HT : (h + 1) * HT],
                            in_=at_d[:, t * TCOLS + h * HT : t * TCOLS + (h + 1) * HT],
                        )
                    o_sb = o_pool.tile([D, TCOLS], F16, tag="o")
                    tiles[t] = (at_sb, o_sb)
                return tiles[t]

            GRP = 2  # chunks per PE stationary group / pipeline lag

            def emit_front(k):
                """G-matmul + DVE stencil multiply for chunk k."""
                at_sb, o_sb = tile_of(k)
                col = (k % cpt) * CHUNK
                g_ps = g_pool.tile([D, CHUNK], F32, tag="g")
                nc.tensor.matmul(
                    out=g_ps[:],
                    lhsT=gt_sb[:],
                    rhs=at_sb[:, col : col + CHUNK],
                    start=True,
                    stop=True,
                )
                # x_dev[p] = g[p+1]*a[p]: rotation baked into G_rot, so this
                # is a single partition-aligned multiply.
                x_ps = x_pool.tile([D, CHUNK], F32, tag="x")
                nc.vector.tensor_mul(
                    out=x_ps[:], in0=g_ps[:], in1=at_sb[:, col : col + CHUNK]
                )
                st[k] = (at_sb, o_sb, col, x_ps)

            def emit_back(k):
                """W-matmul accumulate + bias evac for chunk k."""
                at_sb, o_sb, col, x_ps = st.pop(k)
                nc.tensor.matmul(
                    out=x_ps[:],
                    lhsT=wt_sb[:],
                    rhs=at_sb[:, col : col + CHUNK],
                    start=False,
                    stop=True,
                    skip_group_check=True,
                )
                # out = x + mm + b_lin (per-partition bias), PSUM -> SBUF fp16
                nc.scalar.add(
                    out=o_sb[:, col : col + CHUNK], in_=x_ps[:], add=b_sb[:, 0:1]
                )
                t, c = divmod(k, cpt)
                # half-tile stores; tapered pieces on the last tile so the
                # final transfer (and its ~2us HBM receipt) is small
                if t == ntiles - 1:
                    pieces = {3: (0, 4), 7: (4, 4), 11: (8, 4), 13: (12, 2), 15: (14, 2)}
                else:
                    pieces = {cpt // 2 - 1: (0, cpt // 2), cpt - 1: (cpt // 2, cpt // 2)}
                if c in pieces:
                    # issue immediately: the GpSimd queue is dedicated to
                    # stores, so its semaphore wait delays nothing
                    c0, w = pieces[c]
                    nc.gpsimd.dma_start(
                        out=out_d[
                            :,
                            t * TCOLS + c0 * CHUNK : t * TCOLS + (c0 + w) * CHUNK,
                        ],
                        in_=o_sb[:, c0 * CHUNK : (c0 + w) * CHUNK],
                    )

            # 2-chunk groups, software-pipelined by one group: PE stream is
            # [G(k),G(k+1),W(k-2),W(k-1)] so the PE never waits on the DVE
            # round-trip and stationary reloads amortize over the group.
            for k0 in range(0, nchunks + GRP, GRP):
                for k in range(k0, k0 + GRP):
                    if k < nchunks:
                        emit_front(k)
                for k in range(k0 - GRP, k0):
                    if 0 <= k < nchunks:
                        emit_back(k)

    nc.compile()
    return nc


def make_consts(w: np.ndarray, W_lin: np.ndarray, b_lin: np.ndarray):
    """Host-side constant preparation (all tiny)."""
    w = np.asarray(w, np.float64)
    c1 = w[:, 0] * w[:, 2]
    c2 = w[:, 1] * w[:, 2]
    # column 1 uses w[1,0] as the outer factor (faithful to source)
    c1[1] = w[1, 0] * w[1, 0]
    c2[1] = w[1, 1] * w[1, 0]

    j = np.arange(D)
    G = np.zeros((D, D), np.float64)
    G[j, (j + 1) % D] += c1
    G[j, (j - 2) % D] -= c2

    # Row-rotate everything by -1 so partition p of the device result holds
    # output feature (p+1) mod D; the host un-rotates on assembly.
    G_rot = np.roll(G, -1, axis=0)
    W_rot = np.roll(np.asarray(W_lin, np.float64), -1, axis=0)
    b_rot = np.roll(np.asarray(b_lin, np.float32), -1)
    gt = np.ascontiguousarray(G_rot.T).astype(np.float16)  # lhsT for g_rot
    wt = np.ascontiguousarray(W_rot.T).astype(np.float16)  # lhsT for mm_rot
    b = b_rot.reshape(D, 1)
    return {"gt": gt, "wt": wt, "b": b}


_PROGRAM_CACHE: dict[int, object] = {}
TRACE = False      # test-only: capture NTFF profile on the next kernel() call
TRACE_DIR = None   # test-only: where to keep NTFF/perfetto artifacts
LAST_RESULT = None  # test-only: BassKernelResults of the last run


def _get_program(ncols: int):
    if ncols not in _PROGRAM_CACHE:
        _PROGRAM_CACHE[ncols] = build_program(ncols)
    return _PROGRAM_CACHE[ncols]


def kernel(**inputs) -> np.ndarray:
    inp = np.asarray(inputs["inp"])
    w = np.asarray(inputs["w"], np.float32)
    W_lin = np.asarray(inputs["W_lin"], np.float32)
    b_lin = np.asarray(inputs["b_lin"], np.float32)

    B = inp.shape[0]
    assert inp.shape[1] == D and B % N_CORES == 0
    ncols = B // N_CORES  # original rows per core = device free-dim columns

    consts = make_consts(w, W_lin, b_lin)
    inp16 = inp.astype(np.float16)
    shards = inp16.reshape(N_CORES, ncols, D)

    nc = _get_program(ncols)
    in_maps = [
        {"at": np.ascontiguousarray(shards[i].T), **consts} for i in range(N_CORES)
    ]
    res = run_bass_kernel_spmd(
        nc, in_maps, list(range(N_CORES)), trace=TRACE, tmpdir=TRACE_DIR
    )
    global LAST_RESULT
    LAST_RESULT = res

    out = np.empty((B, D), np.float32)
    for i in range(N_CORES):
        # un-rotate: device partition p holds output feature (p+1) mod D
        out[i * ncols : (i + 1) * ncols] = np.roll(res.results[i]["out"], 1, axis=0).T
    return out


if __name__ == "__main__":
    # quick smoke test on random data vs numpy
    rng = np.random.default_rng(0)
    B = N_CORES * TCOLS * 2
    inp = rng.standard_normal((B, D)).astype(np.float32)
    w = rng.random((D, 3)).astype(np.float32)
    W_lin = (rng.standard_normal((D, D)) / np.sqrt(D)).astype(np.float32)
    b_lin = (rng.standard_normal(D) * 0.01).astype(np.float32)
    dt = np.ones(1, np.float32)

    actual = kernel(inp=inp, dt=dt, w=w, W_lin=W_lin, b_lin=b_lin)

    a = inp.astype(np.float64)
    c1 = (w[:, 0] * w[:, 2]).astype(np.float64)
    c2 = (w[:, 1] * w[:, 2]).astype(np.float64)
    c1[1] = float(w[1, 0]) * float(w[1, 0])
    c2[1] = float(w[1, 1]) * float(w[1, 0])
    ap1 = np.roll(a, -1, 1)
    am2 = np.roll(a, 2, 1)
    am1 = np.roll(a, 1, 1)
    x = (c1 * ap1 - c2 * am2) * am1
    expected = x + a @ W_lin.astype(np.float64).T + b_lin
    err = np.abs(actual - expected).max() / np.abs(expected).max()
    print("scale-relative absmax err:", err)


# revision 57
# speedup vs baseline: 1.0226x; 1.0226x over previous
"""Trainium2 Bass kernel for the BINN convnet problem (fp16, transposed layout).

Computation (per row b of inp, all column indices mod D=128):
    g[b, j]  = c1[j] * a[b, j+1] - c2[j] * a[b, j-2]
    x[b, j]  = g[b, j] * a[b, j-1]
    out      = x + a @ W_lin.T + b_lin
with c1[j] = w[j,0]*w[j,2], c2[j] = w[j,1]*w[j,2], except j==1 where the
outer factor is w[1,0] instead of w[1,2].  g is linear in a: g = a @ G.T for
a constant banded G.

The correctness gate is scale-relative absmax < 2e-2; fp16 end-to-end
measures ~7e-4, so all HBM traffic runs at 2 bytes/elem — half the fp32
baseline (32 MiB/core instead of 64 MiB/core; DMA is the roofline).

Strategy: pure data parallel across 8 NeuronCores (batch split).  The host
pre-transposes each 65536-row shard to A^T [128, 65536] fp16, so on-device:

  1. Loads are plain contiguous DMAs (16 KiB per partition per tile),
     no transposes anywhere on device.
  2. g^T = G @ A^T and mm^T = W_lin @ A^T are matmuls with *constant*
     stationary operands (G^T, W_lin^T) and A^T chunks moving, N=512 per
     PSUM bank, natural transposed output layout.
  3. The stencil roll j-1 is absorbed into the constants: the device
     computes the row-rotated output out_dev[p] = out[(p+1) mod 128] using
     rolled G, W_lin, b_lin, so the DVE multiply x_dev = g_rot * A^T is
     perfectly partition-aligned (PSUM partition offsets are illegal) and
     needs no wrap op.  The host un-rotates for free on assembly.
  4. The W matmul accumulates mm on top of x in PSUM (start=False).
     PSUM "zero pending" bits make a bank's first matmul write after
     start=True overwrite instead of accumulate; since the x banks never
     see start=True, their initial pending state is whatever the previous
     NEFF left, which corrupted each bank's first chunk.  A dummy
     full-region start=True matmul per x bank at init clears the bits.
  5. ScalarE evacuates PSUM -> SBUF fp16 adding b_lin, which in transposed
     layout is a per-partition activation bias.
  6. fp16 stores; the host transposes back and upcasts to fp32.  Stores are
     emitted on the ACT queue two 2-chunk groups late so their semaphore
     wait never stalls ACT's queue ahead of pending evacuations.

PE sees [G,G,W,W] per 2-chunk group - one stationary reload per operand
per group; GpSimd stays idle (its fp16 SBUF add measured only ~58 G
elem/s, which made it the bottleneck when it held the final add).
"""

import os
import sys

import numpy as np

if os.path.isdir("/opt/trn_rl_repo") and "/opt/trn_rl_repo" not in sys.path:
    sys.path.insert(0, "/opt/trn_rl_repo")

import concourse.mybir as mybir
import concourse.tile as tile
from concourse import bacc
from concourse.bass_utils import run_bass_kernel_spmd

D = 128          # feature dim
N_CORES = 8
CHUNK = 512      # columns (= batch rows) per PSUM bank / matmul
TCOLS = 8192     # columns per DMA tile (2 MiB fp16)
F16 = mybir.dt.float16
F32 = mybir.dt.float32


def build_program(ncols: int):
    """Build the single-core Bass program (SPMD across cores).

    ncols = rows of the original problem handled by this core; the device
    works on A^T [128, ncols] fp16.
    """
    assert ncols % TCOLS == 0
    ntiles = ncols // TCOLS
    cpt = TCOLS // CHUNK          # chunks per tile (16)
    nchunks = ntiles * cpt

    nc = bacc.Bacc("TRN2", debug=False, target_bir_lowering=False)

    at_d = nc.declare_dram_parameter("at", [D, ncols], F16, isOutput=False)
    gt_d = nc.declare_dram_parameter("gt", [D, D], F16, isOutput=False)
    wt_d = nc.declare_dram_parameter("wt", [D, D], F16, isOutput=False)
    b_d = nc.declare_dram_parameter("b", [D, 1], F32, isOutput=False)
    out_d = nc.declare_dram_parameter("out", [D, ncols], F16, isOutput=True)

    with tile.TileContext(nc) as tc:
        HT = TCOLS // 2  # loads/stores split in 1 MiB halves (ramp/tail)
        with (
            tc.tile_pool(name="const", bufs=1) as const_pool,
            tc.tile_pool(name="a_sb", bufs=4) as a_pool,
            tc.tile_pool(name="o_sb", bufs=4) as o_pool,
            tc.tile_pool(name="g_ps", bufs=4, space="PSUM") as g_pool,
            tc.tile_pool(name="x_ps", bufs=4, space="PSUM") as x_pool,
        ):
            gt_sb = const_pool.tile([D, D], F16)
            wt_sb = const_pool.tile([D, D], F16)
            b_sb = const_pool.tile([D, 1], F32)
            dum_sb = const_pool.tile([1, D + CHUNK], F16)
            # memset on the otherwise-idle GpSimd queue: it is ready ~1 us
            # before DVE's first post-preamble slot, and the warmup matmuls
            # below gate G(0) on the PE queue
            nc.gpsimd.memset(dum_sb[:], 0.0)
            # The first input piece is the head-of-pipeline gate (transfer +
            # ~2us HBM receipt): issue a small first piece FIRST on the SP
            # queue, ahead of the tiny const DMAs, then ramp piece sizes.
            # SP issue order tuned so each piece's landing (issue + transfer
            # + ~2us HBM receipt) leads the 622ns/chunk demand schedule:
            # gt before G(0)~10.5, wt before W(0)~13.9, b before evac(0)~14.3,
            # doubling piece sizes just ahead of their consuming chunks.
            at0_sb = a_pool.tile([D, TCOLS], F16, tag="at")

            def load0(lo, hi):
                nc.sync.dma_start(out=at0_sb[:, lo:hi], in_=at_d[:, lo:hi])

            # Mid-ramp pieces issue from the ACT HWDGE ring, which is idle
            # between its table load (~8us) and the first evacuation
            # (~13.6us); waitless loads there cannot block anything, and
            # dual-queue issue doubles the ramp's supply rate.
            load0(0, 512)
            nc.sync.dma_start(out=gt_sb[:], in_=gt_d[:, :])
            load0(512, 1024)
            nc.scalar.dma_start(out=at0_sb[:, 1024:2048], in_=at_d[:, 1024:2048])
            nc.scalar.dma_start(out=at0_sb[:, 2048:3072], in_=at_d[:, 2048:3072])
            nc.scalar.dma_start(out=at0_sb[:, 3072:4096], in_=at_d[:, 3072:4096])
            nc.sync.dma_start(out=wt_sb[:], in_=wt_d[:, :])
            nc.sync.dma_start(out=b_sb[:], in_=b_d[:, :])
            load0(4096, 6144)
            load0(6144, 8192)
            # hoist ScalarE's lazy activation-table load out of the pipeline
            warm_sb = const_pool.tile([1, 1], F32)
            nc.scalar.add(out=warm_sb[:], in_=b_sb[0:1, 0:1], add=b_sb[0:1, 0:1])

            # Clear the x banks' PSUM zero-pending bits: one full-region
            # start=True matmul per bank (values are overwritten later).
            # Both operands come from the memset tile so the warmups never
            # wait on the constant DMAs.
            for _ in range(4):
                x_ps = x_pool.tile([D, CHUNK], F32, tag="x")
                nc.tensor.matmul(
                    out=x_ps[:],
                    lhsT=dum_sb[0:1, 0:D],
                    rhs=dum_sb[0:1, D : D + CHUNK],
                    start=True,
                    stop=True,
                )

            tiles = {}  # tile t -> (at_sb, o_sb)
            st = {}     # chunk k -> (at_sb, o_sb, col, x_ps)

            o0_sb = o_pool.tile([D, TCOLS], F16, tag="o")
            tiles[0] = (at0_sb, o0_sb)

            def tile_of(k):
                t, c = divmod(k, cpt)
                if c == 0 and t not in tiles:
                    at_sb = a_pool.tile([D, TCOLS], F16, tag="at")
                    for h in range(2):
                        nc.sync.dma_start(
                            out=at_sb[:, h * HT : (h + 1) * HT],
                            in_=at_d[:, t * TCOLS + h * HT : t * TCOLS + (h + 1) * HT],
                        )
                    o_sb = o_pool.tile([D, TCOLS], F16, tag="o")
                    tiles[t] = (at_sb, o_sb)
                return tiles[t]

            GRP = 2  # chunks per PE stationary group / pipeline lag

            def emit_front(k):
                """G-matmul + DVE stencil multiply for chunk k."""
                at_sb, o_sb = tile_of(k)
                col = (k % cpt) * CHUNK
                g_ps = g_pool.tile([D, CHUNK], F32, tag="g")
                nc.tensor.matmul(
                    out=g_ps[:],
                    lhsT=gt_sb[:],
                    rhs=at_sb[:, col : col + CHUNK],
                    start=True,
                    stop=True,
                )
                # x_dev[p] = g[p+1]*a[p]: rotation baked into G_rot, so this
                # is a single partition-aligned multiply.
                x_ps = x_pool.tile([D, CHUNK], F32, tag="x")
                nc.vector.tensor_mul(
                    out=x_ps[:], in0=g_ps[:], in1=at_sb[:, col : col + CHUNK]
                )
                st[k] = (at_sb, o_sb, col, x_ps)

            def emit_back(k):
                """W-matmul accumulate + bias evac for chunk k."""
                at_sb, o_sb, col, x_ps = st.pop(k)
                nc.tensor.matmul(
                    out=x_ps[:],
                    lhsT=wt_sb[:],
                    rhs=at_sb[:, col : col + CHUNK],
                    start=False,
                    stop=True,
                    skip_group_check=True,
                )
                # out = x + mm + b_lin (per-partition bias), PSUM -> SBUF fp16
                nc.scalar.add(
                    out=o_sb[:, col : col + CHUNK], in_=x_ps[:], add=b_sb[:, 0:1]
                )
                t, c = divmod(k, cpt)
                # half-tile stores; tapered pieces on the last tile so the
                # final transfer (and its ~2us HBM receipt) is small
                if t == ntiles - 1:
                    pieces = {3: (0, 4), 7: (4, 4), 11: (8, 4), 13: (12, 2), 15: (14, 2)}
                else:
                    pieces = {cpt // 2 - 1: (0, cpt // 2), cpt - 1: (cpt // 2, cpt // 2)}
                if c in pieces:
                    # issue immediately: the GpSimd queue is dedicated to
                    # stores, so its semaphore wait delays nothing
                    c0, w = pieces[c]
                    nc.gpsimd.dma_start(
                        out=out_d[
                            :,
                            t * TCOLS + c0 * CHUNK : t * TCOLS + (c0 + w) * CHUNK,
                        ],
                        in_=o_sb[:, c0 * CHUNK : (c0 + w) * CHUNK],
                    )

            # 2-chunk groups, software-pipelined by one group: PE stream is
            # [G(k),G(k+1),W(k-2),W(k-1)] so the PE never waits on the DVE
            # round-trip and stationary reloads amortize over the group.
            for k0 in range(0, nchunks + GRP, GRP):
                for k in range(k0, k0 + GRP):
                    if k < nchunks:
                        emit_front(k)
                for k in range(k0 - GRP, k0):
                    if 0 <= k < nchunks:
                        emit_back(k)

    nc.compile()
    return nc


def make_consts(w: np.ndarray, W_lin: np.ndarray, b_lin: np.ndarray):
    """Host-side constant preparation (all tiny)."""
    w = np.asarray(w, np.float64)
    c1 = w[:, 0] * w[:, 2]
    c2 = w[:, 1] * w[:, 2]
    # column 1 uses w[1,0] as the outer factor (faithful to source)
    c1[1] = w[1, 0] * w[1, 0]
    c2[1] = w[1, 1] * w[1, 0]

    j = np.arange(D)
    G = np.zeros((D, D), np.float64)
    G[j, (j + 1) % D] += c1
    G[j, (j - 2) % D] -= c2

    # Row-rotate everything by -1 so partition p of the device result holds
    # output feature (p+1) mod D; the host un-rotates on assembly.
    G_rot = np.roll(G, -1, axis=0)
    W_rot = np.roll(np.asarray(W_lin, np.float64), -1, axis=0)
    b_rot = np.roll(np.asarray(b_lin, np.float32), -1)
    gt = np.ascontiguousarray(G_rot.T).astype(np.float16)  # lhsT for g_rot
    wt = np.ascontiguousarray(W_rot.T).astype(np.float16)  # lhsT for mm_rot
    b = b_rot.reshape(D, 1)
    return {"gt": gt, "wt": wt, "b": b}


_PROGRAM_CACHE: dict[int, object] = {}
TRACE = False      # test-only: capture NTFF profile on the next kernel() call
TRACE_DIR = None   # test-only: where to keep NTFF/perfetto artifacts
LAST_RESULT = None  # test-only: BassKernelResults of the last run


def _get_program(ncols: int):
    if ncols not in _PROGRAM_CACHE:
        _PROGRAM_CACHE[ncols] = build_program(ncols)
    return _PROGRAM_CACHE[ncols]


def kernel(**inputs) -> np.ndarray:
    inp = np.asarray(inputs["inp"])
    w = np.asarray(inputs["w"], np.float32)
    W_lin = np.asarray(inputs["W_lin"], np.float32)
    b_lin = np.asarray(inputs["b_lin"], np.float32)

    B = inp.shape[0]
    assert inp.shape[1] == D and B % N_CORES == 0
    ncols = B // N_CORES  # original rows per core = device free-dim columns

    consts = make_consts(w, W_lin, b_lin)
    inp16 = inp.astype(np.float16)
    shards = inp16.reshape(N_CORES, ncols, D)

    nc = _get_program(ncols)
    in_maps = [
        {"at": np.ascontiguousarray(shards[i].T), **consts} for i in range(N_CORES)
    ]
    res = run_bass_kernel_spmd(
        nc, in_maps, list(range(N_CORES)), trace=TRACE, tmpdir=TRACE_DIR
    )
    global LAST_RESULT
    LAST_RESULT = res

    out = np.empty((B, D), np.float32)
    for i in range(N_CORES):
        # un-rotate: device partition p holds output feature (p+1) mod D
        out[i * ncols : (i + 1) * ncols] = np.roll(res.results[i]["out"], 1, axis=0).T
    return out


if __name__ == "__main__":
    # quick smoke test on random data vs numpy
    rng = np.random.default_rng(0)
    B = N_CORES * TCOLS * 2
    inp = rng.standard_normal((B, D)).astype(np.float32)
    w = rng.random((D, 3)).astype(np.float32)
    W_lin = (rng.standard_normal((D, D)) / np.sqrt(D)).astype(np.float32)
    b_lin = (rng.standard_normal(D) * 0.01).astype(np.float32)
    dt = np.ones(1, np.float32)

    actual = kernel(inp=inp, dt=dt, w=w, W_lin=W_lin, b_lin=b_lin)

    a = inp.astype(np.float64)
    c1 = (w[:, 0] * w[:, 2]).astype(np.float64)
    c2 = (w[:, 1] * w[:, 2]).astype(np.float64)
    c1[1] = float(w[1, 0]) * float(w[1, 0])
    c2[1] = float(w[1, 1]) * float(w[1, 0])
    ap1 = np.roll(a, -1, 1)
    am2 = np.roll(a, 2, 1)
    am1 = np.roll(a, 1, 1)
    x = (c1 * ap1 - c2 * am2) * am1
    expected = x + a @ W_lin.astype(np.float64).T + b_lin
    err = np.abs(actual - expected).max() / np.abs(expected).max()
    print("scale-relative absmax err:", err)
